# revision 19
# baseline (speedup 1.0000x reference)
"""GPT-2 style multi-head attention on 8 Trainium2 cores (Bass/Tile).

Problem: B=2, T=2048, C=1024, H=16 heads, D=64, fp32 in/out.

Sharding (hardcoded): 2 groups x 4 cores; group g handles batch b=g.
Within a group, rank r computes heads [4r, 4r+4) (tensor parallel over
heads: c_attn column slices), then AllGather of y^T across the group,
then each core computes a 256-column slice of the output projection
(c_proj column slice) plus bias.

Datapath is bf16 (inputs converted on host): all matmul operands are
bf16 with fp32 PSUM accumulation, DVE elementwise runs at 2x on 16-bit,
and weight/x/collective DMA bytes are halved vs fp32.  The softmax
denominator reciprocal + broadcast stays fp32/f32r.  Measured rel err vs
the fp32 reference ~4e-3, well inside the 2e-2 gate.

Kernel dataflow per core, fully pipelined over 512-row t-blocks:
  per t-block tb:
    stage 1: x[tb] -> (PE transpose) x^T;  qk^T[.,tb] = W_qk^T @ x^T
             (per-partition f32 bias on PSUM copyback); V[tb] = x @ W_v
             (bias via ones-row matmul into the accumulating PSUM).  V is
             stored per 128-row k-tile with an appended ones column so
             the AV matmul also emits the softmax denominator for free.
    stage 2 (q block qb=tb): per head pair (even/odd heads share qkT
             chunks on partition bases 0/64, so their QK matmuls run on
             disjoint PE row strips): scores^T[k,q] = K^T.T @ Q^T in
             PSUM -> exp(0.125*s) on ACT -> causal mask multiply
             (diagonal tiles only) -> y_aug^T[65,512] += V_aug^T @ exp^T
             over k tiles (row 64 = sum of exp).  Normalize: reciprocal
             of row 64, ones-matmul broadcast, multiply -> bf16 y slice.
    stage 3: per head pair, AllGather the [128,512] y^T slice across the
             4-core group (two half-size collectives per t-block, so the
             second one of block tb overlaps block tb+1's compute), then
             out[:, col slice] = y^T.T @ W_p slice + bias for the
             previous t-block.  W_p rows are permuted on the host so the
             two gathered halves are contiguous contraction chunks.
"""

import numpy as np
import ml_dtypes

import concourse.bass as bass
import concourse.mybir as mybir
import concourse.tile as tile
from concourse import bacc

P = 128
B, T_FULL, C, H, D = 2, 2048, 1024, 16, 64
F32 = mybir.dt.float32
F32R = mybir.dt.float32r
BF16 = mybir.dt.bfloat16
NP_BF16 = ml_dtypes.bfloat16
EXP = mybir.ActivationFunctionType.Exp
LN = mybir.ActivationFunctionType.Ln
ADD = mybir.AluOpType.add
MUL = mybir.AluOpType.mult
BYPASS = mybir.AluOpType.bypass


class Cfg:
    def __init__(self, n_cores, group_size, T, fake_collective=False,
                 repeat=1, xt_bufs=3, e_bufs=8, x_bufs=4, n_bufs=3):
        self.fake_collective = fake_collective
        self.repeat = repeat
        self.xt_bufs = xt_bufs
        self.e_bufs = e_bufs
        self.x_bufs = x_bufs
        self.n_bufs = n_bufs
        self.n_cores = n_cores
        self.GS = group_size               # cores per batch group
        self.T = T                         # sequence length handled per core
        self.HL = H // group_size          # heads per core
        assert self.HL % 2 == 0
        self.NP = C // group_size          # output-projection columns per core
        self.CC = C // P                   # contraction chunks (8)
        self.TB = T // 512                 # t-blocks == q blocks
        self.QB = T // 512
        self.KT = T // P                   # k tiles
        self.QKCH = self.HL                # qk^T partition chunks (Q | K)
        self.VW = 68                       # per-head V stride: 64 V + 1 ones
        self.HP = self.HL // 2             # head pairs == AG halves per block
        if n_cores == 8:
            self.replica_groups = [[0, 1, 2, 3], [4, 5, 6, 7]]
        elif n_cores == 4:
            self.replica_groups = [[0, 1], [2, 3]]
        elif n_cores == 1:
            self.replica_groups = [[0]]
        else:
            raise ValueError(n_cores)


CFG_FULL = Cfg(8, 4, T_FULL)


def emit(tc, outs, ins, cfg):
    """Emit the SPMD program. outs/ins are dicts of DRAM APs."""
    for rep in range(cfg.repeat):
        _emit_once(tc, outs["out"], ins, cfg, rep)


def _emit_once(tc, out, ins, cfg, rep):
    nc = tc.nc
    GS, T, HL, NP, CC, VW = cfg.GS, cfg.T, cfg.HL, cfg.NP, cfg.CC, cfg.VW
    QKCH = cfg.QKCH

    x = ins["x"]              # [T, C] bf16
    wqk = ins["wqk"]          # [C, HL*128] bf16  (Q cols | K cols)
    wv = ins["wv"]            # [C, HL*64] bf16
    bqk = ins["bqk"]          # [P, HL] f32  (chunk-major per-partition bias)
    bv = ins["bv"]            # [1, HL*64] bf16
    wp = ins["wp"]            # [C, NP] bf16 (rows permuted: AG halves)
    bp = ins["bp"]            # [1, NP] bf16
    masks = ins["masks"]      # [P, 4, 512] bf16
    ident = ins["ident"]      # [P, P] bf16

    from contextlib import ExitStack
    with ExitStack() as _stk:
        persist = _stk.enter_context(tc.tile_pool(name="persist", bufs=1))
        s1 = _stk.enter_context(tc.tile_pool(name="s1", bufs=2))
        s2 = _stk.enter_context(tc.tile_pool(name="s2", bufs=4))
        s3 = _stk.enter_context(tc.tile_pool(name="s3", bufs=2))
        dram = _stk.enter_context(
            tc.tile_pool(name="dram", bufs=1, space="DRAM"))
        ps_acc = _stk.enter_context(tc.tile_pool(
            name="ps_acc", bufs=1, space="PSUM"))
        ps_s = _stk.enter_context(tc.tile_pool(
            name="ps_s", bufs=3, space="PSUM"))
        ps_y = _stk.enter_context(tc.tile_pool(
            name="ps_y", bufs=2, space="PSUM"))
        ps_tp = _stk.enter_context(tc.tile_pool(
            name="ps_tp", bufs=2, space="PSUM"))
        # ---- persistent SBUF tensors ----
        qkT = persist.tile([P, QKCH, T], BF16, tag="qkT")

        def qk_write(m, tb):
            return qkT[:, m, tb * 512:(tb + 1) * 512]

        def qk_q(pb, qch, qb, lo):
            return qkT[pb, qch, qb * 512 + lo:(qb + 1) * 512]

        def qk_k(pb, kch, kt):
            return qkT[pb, kch, kt * P:(kt + 1) * P]

        vsb = persist.tile([P, cfg.KT, HL * VW], BF16, tag="vsb")
        mask_sb = persist.tile([P, 4, 512], BF16, tag="mask")
        ident_sb = persist.tile([P, P], BF16, tag="ident")
        ones_row = persist.tile([1, P], BF16, tag="ones_row")
        wp_sb = persist.tile([P, CC, NP], BF16, tag="wp")
        bp_sb = persist.tile([1, NP], BF16, tag="bp")
        wqk_sb = persist.tile([P, CC, QKCH * P], BF16, tag="wqk")
        wv_sb = persist.tile([P, CC, HL * D], BF16, tag="wv")
        bqk_sb = persist.tile([P, QKCH], F32, tag="bqk")
        bv_sb = persist.tile([1, HL * D], BF16, tag="bv")

        x_r = x.rearrange("(a s p) c -> a p s c", p=P, s=4)
        # prefetch tb0's x tiles ahead of the bulk weight DMAs so the
        # first transposes start immediately.
        x_pre = []
        for ts in range(4):
            xp = s1.tile([P, C], BF16, tag="x", bufs=cfg.x_bufs,
                         name=f"xpre{ts}")
            nc.sync.dma_start(xp[:], x_r[0, :, ts, :])
            x_pre.append(xp)
        # ident first on the sync queue (transposes need it immediately);
        # bulk weight loads go on the gpsimd (SWDGE) queue so the x-tile
        # DMAs on the sync queue aren't stuck behind the weights.
        # spread the startup loads over per-engine DMA queues so the
        # transposes (ident), first qk matmuls (wqk), and attention(0)
        # (masks) aren't serialized behind the x tiles on one ring.
        nc.scalar.dma_start(ident_sb[:], ident)
        nc.scalar.dma_start(
            wqk_sb[:], wqk.rearrange("(c p) m -> p c m", p=P))
        nc.gpsimd.dma_start(
            wv_sb[:], wv.rearrange("(c p) m -> p c m", p=P))
        nc.gpsimd.dma_start(bqk_sb[:], bqk)
        nc.gpsimd.dma_start(bv_sb[:], bv)
        nc.scalar.dma_start(mask_sb[:], masks)
        nc.gpsimd.dma_start(
            wp_sb[:], wp.rearrange("(c p) n -> p c n", p=P))
        nc.gpsimd.dma_start(bp_sb[:], bp)

        # memset can't write f32r/bf16; memset f32 scratch, copy-convert.
        scratch1 = persist.tile([P, max(P, cfg.KT * HL)], F32, tag="scratch1")
        nc.vector.memset(scratch1[:], 1.0)
        nc.vector.tensor_copy(ones_row[:], scratch1[0:1, 0:P])
        # ones columns inside the V tile (col 64 of each head's 68-wide slot)
        vsb_h = vsb.rearrange("p k (h w) -> p k h w", w=VW)
        nc.vector.tensor_copy(
            vsb_h[:, :, :, 64:65],
            scratch1[:, 0:cfg.KT * HL].rearrange(
                "p (k h o) -> p k h o", k=cfg.KT, h=HL, o=1),
        )

        # per (t-block, head-pair) AllGather buffers: in [128, 512] out
        # [GS*128, 512], both bf16.
        ag_in = [
            [dram.tile([2 * D, 512], BF16, tag=f"agin{qb}_{hp}",
                       name=f"agin{qb}_{hp}_{rep}")
             for hp in range(cfg.HP)]
            for qb in range(cfg.QB)
        ]
        ag_out = [
            [dram.tile([GS * 2 * D, 512], BF16, tag=f"agout{qb}_{hp}",
                       name=f"agout{qb}_{hp}_{rep}")
             for hp in range(cfg.HP)]
            for qb in range(cfg.QB)
        ]

        def stage1(tb):
            xT = s1.tile([P, CC, 512], BF16, tag="xT", bufs=cfg.xt_bufs)
            for ts in range(4):
                if tb == 0:
                    x_sb = x_pre[ts]
                else:
                    x_sb = s1.tile([P, C], BF16, tag="x", bufs=cfg.x_bufs)
                    nc.sync.dma_start(x_sb[:], x_r[tb, :, ts, :])
                for cc0 in range(0, CC, 4):
                    tp = ps_tp.tile([P, 512], BF16, tag="tp")
                    for i in range(4):
                        cc = cc0 + i
                        nc.tensor.transpose(
                            tp[:, i * P:(i + 1) * P],
                            x_sb[:, cc * P:(cc + 1) * P], ident_sb[:])
                    nc.vector.tensor_copy(
                        xT[:, cc0:cc0 + 4, ts * P:(ts + 1) * P],
                        tp.rearrange("p (i q) -> p i q", q=P))
            # qk^T: lhsT = W chunk, rhs = x^T chunk
            for m in range(QKCH):
                acc = ps_acc.tile([P, 512], F32, tag="acc")
                for cc in range(CC):
                    nc.tensor.matmul(
                        acc[:],
                        wqk_sb[:, cc, m * P:(m + 1) * P],
                        xT[:, cc, :],
                        start=(cc == 0),
                        stop=(cc == CC - 1),
                    )
                nc.vector.tensor_scalar_add(
                    qk_write(m, tb), acc[:],
                    bqk_sb[:, m:m + 1],
                )
            # V natural: lhsT = x^T chunk, rhs = W_v
            for ts in range(4):
                kt = tb * 4 + ts
                vp = ps_acc.tile([P, 512], F32, tag="acc")
                for cc in range(CC):
                    nc.tensor.matmul(
                        vp[:, 0:HL * D],
                        xT[:, cc, ts * P:(ts + 1) * P],
                        wv_sb[:, cc, :],
                        start=(cc == 0),
                        stop=False,
                    )
                nc.tensor.matmul(
                    vp[:, 0:HL * D], ones_row[:1, :], bv_sb[:1, :],
                    start=False, stop=True,
                )
                nc.vector.tensor_copy(
                    vsb_h[:, kt, :, 0:64],
                    vp[:, 0:HL * D].rearrange("p (h d) -> p h d", d=D),
                )

        def attention(qb):
            # even/odd head pairs sit on partition bases 0 and 64 of the
            # same qkT chunks; interleaving their QK matmuls puts them on
            # disjoint PE row strips (tile_position auto-derived), so the
            # two 64-contract matmuls run concurrently in the array.
            nkt = 4 * qb + 4
            kt_order = list(range(4 * qb, nkt)) + list(range(0, 4 * qb))
            for hp in range(cfg.HP):
                hs = (2 * hp, 2 * hp + 1)
                qch, kch = hp, QKCH // 2 + hp
                pbs = [slice((h % 2) * 64, (h % 2) * 64 + 64) for h in hs]
                ys = [ps_y.tile([65, 512], F32, tag="y",
                                name=f"y{qb}_{h}") for h in hs]
                for ki, kt in enumerate(kt_order):
                    j = kt - 4 * qb
                    lo = 128 * j if j > 0 else 0
                    ss, es = [], []
                    for i in range(2):
                        s = ps_s.tile([P, 512], F32, tag="s",
                                      name=f"s{qb}_{kt}_{i}")
                        nc.tensor.matmul(
                            s[:, lo:],
                            qk_k(pbs[i], kch, kt),
                            qk_q(pbs[i], qch, qb, lo),
                            start=True, stop=True,
                        )
                        ss.append(s)
                    for i in range(2):
                        e = s2.tile([P, 512], BF16, tag="e",
                                    bufs=cfg.e_bufs,
                                    name=f"e{qb}_{kt}_{i}")
                        nc.scalar.activation(
                            e[:, lo:], ss[i][:, lo:], EXP, scale=0.125)
                        if j >= 0:
                            nc.vector.tensor_mul(
                                e[:, lo:], e[:, lo:], mask_sb[:, j, lo:])
                        es.append(e)
                    for i in range(2):
                        nc.tensor.matmul(
                            ys[i][:, lo:],
                            vsb[:, kt, hs[i] * VW:hs[i] * VW + 65],
                            es[i][:, lo:],
                            start=(ki == 0), stop=(ki == nkt - 1),
                        )
                for i in range(2):
                    _normalize(qb, hp, i, ys[i])
                allgather(qb, hp)

        def _normalize(qb, hp, i, y):
            # 1/den via the fast-approx custom DVE op (~18 bits, one op):
            # the exact DVE reciprocal is single-lane ~3.3us on [1,512],
            # and the Ln/Exp ACT alternative forces ~1.3us activation
            # table reloads between it and the scores exps.
            h = 2 * hp + i
            den = s2.tile([1, 512], F32, tag="den", bufs=cfg.n_bufs,
                          name=f"den{qb}_{h}")
            nc.vector.tensor_copy(den[:], y[64:65, :])
            rec = s2.tile([1, 512], F32, tag="rec", bufs=cfg.n_bufs,
                          name=f"rec{qb}_{h}")
            nc.vector.reciprocal_approx_fast(rec[:], den[:])
            rec_bf = s2.tile([1, 512], BF16, tag="rec_bf", bufs=cfg.n_bufs,
                             name=f"recb{qb}_{h}")
            nc.vector.tensor_copy(rec_bf[:], rec[:])
            bc = ps_s.tile([P, 512], F32, tag="s", name=f"bc{qb}_{h}")
            nc.tensor.matmul(
                bc[0:64, :], ones_row[:1, 0:64], rec_bf[:1, :],
                start=True, stop=True,
            )
            bc_sb = s2.tile([64, 512], F32, tag="bc_sb", bufs=cfg.n_bufs,
                            name=f"bcs{qb}_{h}")
            nc.vector.tensor_copy(bc_sb[:], bc[0:64, :])
            yn = s2.tile([64, 512], BF16, tag="yn", bufs=cfg.n_bufs,
                         name=f"yn{qb}_{h}")
            nc.vector.tensor_mul(yn[:], y[0:64, :], bc_sb[:])
            nc.sync.dma_start(ag_in[qb][hp][i * 64:(i + 1) * 64, :], yn[:])

        def allgather(qb, hp):
            if cfg.fake_collective:
                # timing-model variant (TimelineSim can't run collectives):
                # stand-in DRAM->DRAM copy.
                nc.sync.dma_start(
                    ag_out[qb][hp][0:2 * D, :], ag_in[qb][hp][:])
                return
            nc.gpsimd.collective_compute(
                "AllGather", BYPASS,
                replica_groups=cfg.replica_groups,
                ins=[ag_in[qb][hp].opt()],
                outs=[ag_out[qb][hp].opt()],
            )

        def proj(qb):
            # contraction rows: half 0 = ranks x heads {0,1}, half 1 =
            # ranks x heads {2,3}; wp rows are host-permuted to match.
            # One bulk DMA per gathered half (512 KB streams at full rate)
            # instead of per-t-block strided loads.
            ag_sb = [
                s3.tile([P, CC // 2, 512], BF16, tag=f"ag{hp}",
                        name=f"ag{qb}_{hp}")
                for hp in range(cfg.HP)
            ]
            for hp in range(cfg.HP):
                nc.sync.dma_start(
                    ag_sb[hp][:],
                    ag_out[qb][hp].rearrange("(c p) t -> p c t", p=P))
            for tt in range(4):
                op = ps_acc.tile([P, 512], F32, tag="acc")
                for cc in range(CC):
                    hp, c = divmod(cc, CC // 2)
                    nc.tensor.matmul(
                        op[:, 0:NP],
                        ag_sb[hp][:, c, tt * P:(tt + 1) * P],
                        wp_sb[:, cc, :], start=(cc == 0), stop=False,
                    )
                nc.tensor.matmul(
                    op[:, 0:NP], ones_row[:1, :], bp_sb[:1, :],
                    start=False, stop=True,
                )
                o_sb = s3.tile([P, NP], F32, tag="osb")
                nc.vector.tensor_copy(o_sb[:], op[:, 0:NP])
                row = (qb * 4 + tt) * P
                nc.sync.dma_start(out[row:row + P, :], o_sb[:])

        # fused pipeline: attention(qb) needs exactly the k-tiles stage1(tb)
        # has produced; the AllGathers fire per head pair inside
        # attention(), so the later ones overlap the next block's compute.
        for tb in range(cfg.TB):
            stage1(tb)
            attention(tb)
            if tb > 0:
                proj(tb - 1)
        proj(cfg.TB - 1)


def make_core_inputs(x_full, c_attn_w, c_attn_b, c_proj_w, c_proj_b, cfg,
                     core):
    """Host-side input sharding for one core."""
    GS, HL, NP, T = cfg.GS, cfg.HL, cfg.NP, cfg.T
    g, rk = divmod(core, GS)
    g = g % B  # tolerate more groups than batches (sim configs)
    hs = slice(rk * HL * D, (rk + 1) * HL * D)
    wq = c_attn_w[:, 0 * C:1 * C][:, hs]
    wk = c_attn_w[:, 1 * C:2 * C][:, hs]
    wv = c_attn_w[:, 2 * C:3 * C][:, hs]
    bq = c_attn_b[0 * C:1 * C][hs]
    bk = c_attn_b[1 * C:2 * C][hs]
    bv = c_attn_b[2 * C:3 * C][hs]
    cs = slice(rk * NP, (rk + 1) * NP)

    # c_proj rows permuted to match the gathered layout: half-major,
    # then rank-major, then 2 heads x 64 dims.
    perm = []
    for half in range(HL // 2):
        for r in range(GS):
            base = (r * HL + 2 * half) * D
            perm.extend(range(base, base + 2 * D))
    wp = c_proj_w[np.array(perm)][:, cs]

    pp = np.arange(P)[:, None, None]
    jj = np.arange(4)[None, :, None]
    qq = np.arange(512)[None, None, :]
    masks = (qq >= pp + 128 * jj).astype(NP_BF16)

    bf = NP_BF16
    return {
        "x": np.ascontiguousarray(x_full[g, :T]).astype(bf),
        "wqk": np.ascontiguousarray(
            np.concatenate([wq, wk], axis=1)).astype(bf),
        "wv": np.ascontiguousarray(wv).astype(bf),
        "bqk": np.ascontiguousarray(
            np.concatenate([bq, bk]).reshape(cfg.QKCH, P).T, np.float32),
        "bv": np.ascontiguousarray(bv[None, :]).astype(bf),
        "wp": np.ascontiguousarray(wp).astype(bf),
        "bp": np.ascontiguousarray(c_proj_b[cs][None, :]).astype(bf),
        "masks": masks,
        "ident": np.eye(P).astype(bf),
    }


_CACHE = {}


def _build_full():
    if "nc" in _CACHE:
        return _CACHE["nc"]
    cfg = CFG_FULL
    nc = bacc.Bacc(
        "TRN2", target_bir_lowering=False, debug=False,
        num_devices=cfg.n_cores,
    )
    ins = {}
    shapes = {
        "x": ((cfg.T, C), BF16),
        "wqk": ((C, cfg.QKCH * P), BF16),
        "wv": ((C, cfg.HL * D), BF16),
        "bqk": ((P, cfg.QKCH), F32),
        "bv": ((1, cfg.HL * D), BF16),
        "wp": ((C, cfg.NP), BF16),
        "bp": ((1, cfg.NP), BF16),
        "masks": ((P, 4, 512), BF16),
        "ident": ((P, P), BF16),
    }
    for name, (shape, dt) in shapes.items():
        ins[name] = nc.dram_tensor(
            name, list(shape), dt, kind="ExternalInput").ap()
    outs = {
        "out": nc.dram_tensor(
            "out", [cfg.T, cfg.NP], F32, kind="ExternalOutput").ap()
    }
    with tile.TileContext(nc) as tc:
        emit(tc, outs, ins, cfg)
    nc.compile()
    _CACHE["nc"] = nc
    return nc


def kernel(**inputs):
    from concourse.bass_utils import run_bass_kernel_spmd

    cfg = CFG_FULL
    x = np.asarray(inputs["x"], np.float32)
    c_attn_w = np.asarray(inputs["c_attn_w"], np.float32)
    c_attn_b = np.asarray(inputs["c_attn_b"], np.float32)
    c_proj_w = np.asarray(inputs["c_proj_w"], np.float32)
    c_proj_b = np.asarray(inputs["c_proj_b"], np.float32)

    nc = _build_full()
    in_maps = [
        make_core_inputs(x, c_attn_w, c_attn_b, c_proj_w, c_proj_b, cfg, core)
        for core in range(cfg.n_cores)
    ]
    res = run_bass_kernel_spmd(nc, in_maps, core_ids=list(range(cfg.n_cores)))
    out = np.empty((B, T_FULL, C), np.float32)
    for core in range(cfg.n_cores):
        g, rk = divmod(core, cfg.GS)
        out[g, :, rk * cfg.NP:(rk + 1) * cfg.NP] = res.results[core]["out"]
    return out


# revision 22
# speedup vs baseline: 1.0978x; 1.0978x over previous
"""GPT-2 style multi-head attention on 8 Trainium2 cores (Bass/Tile).

Problem: B=2, T=2048, C=1024, H=16 heads, D=64, fp32 in/out.

Sharding (hardcoded): 2 groups x 4 cores; group g handles batch b=g.
Within a group, rank r computes heads [4r, 4r+4) (tensor parallel over
heads: c_attn column slices), then AllGather of y^T across the group,
then each core computes a 256-column slice of the output projection
(c_proj column slice) plus bias.

Datapath is bf16 (inputs converted on host): all matmul operands are
bf16 with fp32 PSUM accumulation, DVE elementwise runs at 2x on 16-bit,
and weight/x/collective DMA bytes are halved vs fp32.  The softmax
denominator reciprocal + broadcast stays fp32/f32r.  Measured rel err vs
the fp32 reference ~4e-3, well inside the 2e-2 gate.

Kernel dataflow per core, fully pipelined over 512-row t-blocks:
  per t-block tb:
    stage 1: x[tb] -> (PE transpose) x^T;  qk^T[.,tb] = W_qk^T @ x^T
             (per-partition f32 bias on PSUM copyback); V[tb] = x @ W_v
             (bias via ones-row matmul into the accumulating PSUM).  V is
             stored per 128-row k-tile with an appended ones column so
             the AV matmul also emits the softmax denominator for free.
    stage 2 (q block qb=tb): per head pair (even/odd heads share qkT
             chunks on partition bases 0/64, so their QK matmuls run on
             disjoint PE row strips): scores^T[k,q] = K^T.T @ Q^T in
             PSUM -> exp(0.125*s) on ACT -> causal mask multiply
             (diagonal tiles only) -> y_aug^T[65,512] += V_aug^T @ exp^T
             over k tiles (row 64 = sum of exp).  Normalize: reciprocal
             of row 64, ones-matmul broadcast, multiply -> bf16 y slice.
    stage 3: per head pair, AllGather the [128,512] y^T slice across the
             4-core group (two half-size collectives per t-block, so the
             second one of block tb overlaps block tb+1's compute), then
             out[:, col slice] = y^T.T @ W_p slice + bias for the
             previous t-block.  W_p rows are permuted on the host so the
             two gathered halves are contiguous contraction chunks.
"""

import numpy as np
import ml_dtypes

import concourse.bass as bass
import concourse.mybir as mybir
import concourse.tile as tile
from concourse import bacc

P = 128
B, T_FULL, C, H, D = 2, 2048, 1024, 16, 64
F32 = mybir.dt.float32
F32R = mybir.dt.float32r
BF16 = mybir.dt.bfloat16
NP_BF16 = ml_dtypes.bfloat16
EXP = mybir.ActivationFunctionType.Exp
LN = mybir.ActivationFunctionType.Ln
ADD = mybir.AluOpType.add
MUL = mybir.AluOpType.mult
BYPASS = mybir.AluOpType.bypass


class Cfg:
    def __init__(self, n_cores, group_size, T, fake_collective=False,
                 repeat=1, xt_bufs=3, e_bufs=8, x_bufs=4, n_bufs=3):
        self.fake_collective = fake_collective
        self.repeat = repeat
        self.xt_bufs = xt_bufs
        self.e_bufs = e_bufs
        self.x_bufs = x_bufs
        self.n_bufs = n_bufs
        self.n_cores = n_cores
        self.GS = group_size               # cores per batch group
        self.T = T                         # sequence length handled per core
        self.HL = H // group_size          # heads per core
        assert self.HL % 2 == 0
        self.NP = C // group_size          # output-projection columns per core
        self.CC = C // P                   # contraction chunks (8)
        self.TB = T // 512                 # t-blocks == q blocks
        self.QB = T // 512
        self.KT = T // P                   # k tiles
        self.QKCH = self.HL                # qk^T partition chunks (Q | K)
        self.VW = 68                       # per-head V stride: 64 V + 1 ones
        self.HP = self.HL // 2             # head pairs == AG halves per block
        if n_cores == 8:
            self.replica_groups = [[0, 1, 2, 3], [4, 5, 6, 7]]
        elif n_cores == 4:
            self.replica_groups = [[0, 1], [2, 3]]
        elif n_cores == 1:
            self.replica_groups = [[0]]
        else:
            raise ValueError(n_cores)


CFG_FULL = Cfg(8, 4, T_FULL)


def emit(tc, outs, ins, cfg):
    """Emit the SPMD program. outs/ins are dicts of DRAM APs."""
    for rep in range(cfg.repeat):
        _emit_once(tc, outs["out"], ins, cfg, rep)


def _emit_once(tc, out, ins, cfg, rep):
    nc = tc.nc
    GS, T, HL, NP, CC, VW = cfg.GS, cfg.T, cfg.HL, cfg.NP, cfg.CC, cfg.VW
    QKCH = cfg.QKCH

    x = ins["x"]              # [T, C] bf16
    wqk = ins["wqk"]          # [C, HL*128] bf16  (Q cols | K cols)
    wv = ins["wv"]            # [C, HL*64] bf16
    bqk = ins["bqk"]          # [P, HL] f32  (chunk-major per-partition bias)
    bv = ins["bv"]            # [1, HL*64] bf16
    wp = ins["wp"]            # [C, NP] bf16 (rows permuted: AG halves)
    bp = ins["bp"]            # [1, NP] bf16
    masks = ins["masks"]      # [P, 4, 512] bf16
    ident = ins["ident"]      # [P, P] bf16

    from contextlib import ExitStack
    with ExitStack() as _stk:
        persist = _stk.enter_context(tc.tile_pool(name="persist", bufs=1))
        s1 = _stk.enter_context(tc.tile_pool(name="s1", bufs=2))
        s2 = _stk.enter_context(tc.tile_pool(name="s2", bufs=4))
        s3 = _stk.enter_context(tc.tile_pool(name="s3", bufs=2))
        dram = _stk.enter_context(
            tc.tile_pool(name="dram", bufs=1, space="DRAM"))
        ps_acc = _stk.enter_context(tc.tile_pool(
            name="ps_acc", bufs=1, space="PSUM"))
        ps_s = _stk.enter_context(tc.tile_pool(
            name="ps_s", bufs=3, space="PSUM"))
        ps_y = _stk.enter_context(tc.tile_pool(
            name="ps_y", bufs=2, space="PSUM"))
        ps_tp = _stk.enter_context(tc.tile_pool(
            name="ps_tp", bufs=2, space="PSUM"))
        # ---- persistent SBUF tensors ----
        qkT = persist.tile([P, QKCH, T], BF16, tag="qkT")

        def qk_write(m, tb):
            return qkT[:, m, tb * 512:(tb + 1) * 512]

        def qk_q(pb, qch, qb, lo):
            return qkT[pb, qch, qb * 512 + lo:(qb + 1) * 512]

        def qk_k(pb, kch, kt):
            return qkT[pb, kch, kt * P:(kt + 1) * P]

        vsb = persist.tile([P, cfg.KT, HL * VW], BF16, tag="vsb")
        mask_sb = persist.tile([P, 4, 512], BF16, tag="mask")
        ident_sb = persist.tile([P, P], BF16, tag="ident")
        ones_row = persist.tile([1, P], BF16, tag="ones_row")
        wp_sb = persist.tile([P, CC, NP], BF16, tag="wp")
        bp_sb = persist.tile([1, NP], BF16, tag="bp")
        wqk_sb = persist.tile([P, CC, QKCH * P], BF16, tag="wqk")
        wv_sb = persist.tile([P, CC, HL * D], BF16, tag="wv")
        bqk_sb = persist.tile([P, QKCH], F32, tag="bqk")
        bv_sb = persist.tile([1, HL * D], BF16, tag="bv")

        x_r = x.rearrange("(a s p) c -> a p s c", p=P, s=4)
        # prefetch tb0's x tiles ahead of the bulk weight DMAs so the
        # first transposes start immediately.
        x_pre = []
        for ts in range(4):
            xp = s1.tile([P, C], BF16, tag="x", bufs=cfg.x_bufs,
                         name=f"xpre{ts}")
            nc.sync.dma_start(xp[:], x_r[0, :, ts, :])
            x_pre.append(xp)
        # ident first on the sync queue (transposes need it immediately);
        # bulk weight loads go on the gpsimd (SWDGE) queue so the x-tile
        # DMAs on the sync queue aren't stuck behind the weights.
        # spread the startup loads over per-engine DMA queues so the
        # transposes (ident), first qk matmuls (wqk), and attention(0)
        # (masks) aren't serialized behind the x tiles on one ring.
        nc.scalar.dma_start(ident_sb[:], ident)
        nc.scalar.dma_start(
            wqk_sb[:], wqk.rearrange("(c p) m -> p c m", p=P))
        nc.gpsimd.dma_start(
            wv_sb[:], wv.rearrange("(c p) m -> p c m", p=P))
        nc.gpsimd.dma_start(bqk_sb[:], bqk)
        nc.gpsimd.dma_start(bv_sb[:], bv)
        nc.scalar.dma_start(mask_sb[:], masks)
        nc.gpsimd.dma_start(
            wp_sb[:], wp.rearrange("(c p) n -> p c n", p=P))
        nc.gpsimd.dma_start(bp_sb[:], bp)

        # memset can't write f32r/bf16; memset f32 scratch, copy-convert.
        scratch1 = persist.tile([P, max(P, cfg.KT * HL)], F32, tag="scratch1")
        nc.vector.memset(scratch1[:], 1.0)
        nc.vector.tensor_copy(ones_row[:], scratch1[0:1, 0:P])
        # ones columns inside the V tile (col 64 of each head's 68-wide slot)
        vsb_h = vsb.rearrange("p k (h w) -> p k h w", w=VW)
        nc.vector.tensor_copy(
            vsb_h[:, :, :, 64:65],
            scratch1[:, 0:cfg.KT * HL].rearrange(
                "p (k h o) -> p k h o", k=cfg.KT, h=HL, o=1),
        )

        # per (t-block, head-pair) AllGather buffers: in [128, 512] out
        # [GS*128, 512], both bf16.
        ag_in = [
            [dram.tile([2 * D, 512], BF16, tag=f"agin{qb}_{hp}",
                       name=f"agin{qb}_{hp}_{rep}")
             for hp in range(cfg.HP)]
            for qb in range(cfg.QB)
        ]
        ag_out = [
            [dram.tile([GS * 2 * D, 512], BF16, tag=f"agout{qb}_{hp}",
                       name=f"agout{qb}_{hp}_{rep}")
             for hp in range(cfg.HP)]
            for qb in range(cfg.QB)
        ]

        def stage1(tb):
            xT = s1.tile([P, CC, 512], BF16, tag="xT", bufs=cfg.xt_bufs)
            for ts in range(4):
                if tb == 0:
                    x_sb = x_pre[ts]
                else:
                    x_sb = s1.tile([P, C], BF16, tag="x", bufs=cfg.x_bufs)
                    nc.sync.dma_start(x_sb[:], x_r[tb, :, ts, :])
                for cc0 in range(0, CC, 4):
                    tp = ps_tp.tile([P, 512], BF16, tag="tp")
                    for i in range(4):
                        cc = cc0 + i
                        nc.tensor.transpose(
                            tp[:, i * P:(i + 1) * P],
                            x_sb[:, cc * P:(cc + 1) * P], ident_sb[:])
                    nc.vector.tensor_copy(
                        xT[:, cc0:cc0 + 4, ts * P:(ts + 1) * P],
                        tp.rearrange("p (i q) -> p i q", q=P))
            # qk^T: lhsT = W chunk, rhs = x^T chunk
            for m in range(QKCH):
                acc = ps_acc.tile([P, 512], F32, tag="acc")
                for cc in range(CC):
                    nc.tensor.matmul(
                        acc[:],
                        wqk_sb[:, cc, m * P:(m + 1) * P],
                        xT[:, cc, :],
                        start=(cc == 0),
                        stop=(cc == CC - 1),
                    )
                nc.vector.tensor_scalar_add(
                    qk_write(m, tb), acc[:],
                    bqk_sb[:, m:m + 1],
                )
            # V natural: lhsT = x^T chunk, rhs = W_v
            for ts in range(4):
                kt = tb * 4 + ts
                vp = ps_acc.tile([P, 512], F32, tag="acc")
                for cc in range(CC):
                    nc.tensor.matmul(
                        vp[:, 0:HL * D],
                        xT[:, cc, ts * P:(ts + 1) * P],
                        wv_sb[:, cc, :],
                        start=(cc == 0),
                        stop=False,
                    )
                nc.tensor.matmul(
                    vp[:, 0:HL * D], ones_row[:1, :], bv_sb[:1, :],
                    start=False, stop=True,
                )
                nc.vector.tensor_copy(
                    vsb_h[:, kt, :, 0:64],
                    vp[:, 0:HL * D].rearrange("p (h d) -> p h d", d=D),
                )

        def attention(qb):
            # even/odd head pairs sit on partition bases 0 and 64 of the
            # same qkT chunks; interleaving their QK matmuls puts them on
            # disjoint PE row strips (tile_position auto-derived), so the
            # two 64-contract matmuls run concurrently in the array.
            nkt = 4 * qb + 4
            kt_order = list(range(4 * qb, nkt)) + list(range(0, 4 * qb))
            for hp in range(cfg.HP):
                hs = (2 * hp, 2 * hp + 1)
                qch, kch = hp, QKCH // 2 + hp
                pbs = [slice((h % 2) * 64, (h % 2) * 64 + 64) for h in hs]
                ys = [ps_y.tile([65, 512], F32, tag="y",
                                name=f"y{qb}_{h}") for h in hs]
                for ki, kt in enumerate(kt_order):
                    j = kt - 4 * qb
                    lo = 128 * j if j > 0 else 0
                    ss, es = [], []
                    for i in range(2):
                        s = ps_s.tile([P, 512], F32, tag="s",
                                      name=f"s{qb}_{kt}_{i}")
                        nc.tensor.matmul(
                            s[:, lo:],
                            qk_k(pbs[i], kch, kt),
                            qk_q(pbs[i], qch, qb, lo),
                            start=True, stop=True,
                        )
                        ss.append(s)
                    for i in range(2):
                        e = s2.tile([P, 512], BF16, tag="e",
                                    bufs=cfg.e_bufs,
                                    name=f"e{qb}_{kt}_{i}")
                        nc.scalar.activation(
                            e[:, lo:], ss[i][:, lo:], EXP, scale=0.125)
                        if j >= 0:
                            nc.vector.tensor_mul(
                                e[:, lo:], e[:, lo:], mask_sb[:, j, lo:])
                        es.append(e)
                    for i in range(2):
                        nc.tensor.matmul(
                            ys[i][:, lo:],
                            vsb[:, kt, hs[i] * VW:hs[i] * VW + 65],
                            es[i][:, lo:],
                            start=(ki == 0), stop=(ki == nkt - 1),
                        )
                for i in range(2):
                    _normalize(qb, hp, i, ys[i])
                allgather(qb, hp)

        def _normalize(qb, hp, i, y):
            # 1/den via the fast-approx custom DVE op (~18 bits, one op):
            # the exact DVE reciprocal is single-lane ~3.3us on [1,512],
            # and the Ln/Exp ACT alternative forces ~1.3us activation
            # table reloads between it and the scores exps.
            h = 2 * hp + i
            den = s2.tile([1, 512], F32, tag="den", bufs=cfg.n_bufs,
                          name=f"den{qb}_{h}")
            nc.vector.tensor_copy(den[:], y[64:65, :])
            rec = s2.tile([1, 512], F32, tag="rec", bufs=cfg.n_bufs,
                          name=f"rec{qb}_{h}")
            nc.vector.reciprocal_approx_fast(rec[:], den[:])
            rec_bf = s2.tile([1, 512], BF16, tag="rec_bf", bufs=cfg.n_bufs,
                             name=f"recb{qb}_{h}")
            nc.vector.tensor_copy(rec_bf[:], rec[:])
            bc = ps_s.tile([P, 512], F32, tag="s", name=f"bc{qb}_{h}")
            nc.tensor.matmul(
                bc[0:64, :], ones_row[:1, 0:64], rec_bf[:1, :],
                start=True, stop=True,
            )
            bc_sb = s2.tile([64, 512], F32, tag="bc_sb", bufs=cfg.n_bufs,
                            name=f"bcs{qb}_{h}")
            nc.vector.tensor_copy(bc_sb[:], bc[0:64, :])
            yn = s2.tile([64, 512], BF16, tag="yn", bufs=cfg.n_bufs,
                         name=f"yn{qb}_{h}")
            nc.vector.tensor_mul(yn[:], y[0:64, :], bc_sb[:])
            nc.sync.dma_start(ag_in[qb][hp][i * 64:(i + 1) * 64, :], yn[:])

        def allgather(qb, hp):
            if cfg.fake_collective:
                # timing-model variant (TimelineSim can't run collectives):
                # stand-in DRAM->DRAM copy.
                nc.sync.dma_start(
                    ag_out[qb][hp][0:2 * D, :], ag_in[qb][hp][:])
                return
            nc.gpsimd.collective_compute(
                "AllGather", BYPASS,
                replica_groups=cfg.replica_groups,
                ins=[ag_in[qb][hp].opt()],
                outs=[ag_out[qb][hp].opt()],
            )

        def proj(qb):
            # contraction rows: half 0 = ranks x heads {0,1}, half 1 =
            # ranks x heads {2,3}; wp rows are host-permuted to match.
            # One bulk DMA per gathered half (512 KB streams at full rate)
            # instead of per-t-block strided loads.
            ag_sb = [
                s3.tile([P, CC // 2, 512], BF16, tag=f"ag{hp}",
                        name=f"ag{qb}_{hp}")
                for hp in range(cfg.HP)
            ]
            # scalar queue: an ag load waits on its AllGather semaphore,
            # and on the sync queue that would head-of-line block the
            # x-tile and yn DMAs of later blocks.
            for hp in range(cfg.HP):
                nc.scalar.dma_start(
                    ag_sb[hp][:],
                    ag_out[qb][hp].rearrange("(c p) t -> p c t", p=P))
            for tt in range(4):
                op = ps_acc.tile([P, 512], F32, tag="acc")
                for cc in range(CC):
                    hp, c = divmod(cc, CC // 2)
                    nc.tensor.matmul(
                        op[:, 0:NP],
                        ag_sb[hp][:, c, tt * P:(tt + 1) * P],
                        wp_sb[:, cc, :], start=(cc == 0), stop=False,
                    )
                nc.tensor.matmul(
                    op[:, 0:NP], ones_row[:1, :], bp_sb[:1, :],
                    start=False, stop=True,
                )
                o_sb = s3.tile([P, NP], F32, tag="osb")
                nc.vector.tensor_copy(o_sb[:], op[:, 0:NP])
                row = (qb * 4 + tt) * P
                nc.scalar.dma_start(out[row:row + P, :], o_sb[:])

        # fused pipeline: attention(qb) needs exactly the k-tiles stage1(tb)
        # has produced; the AllGathers fire per head pair inside
        # attention(), so the later ones overlap the next block's compute.
        # proj is deferred by TWO blocks: the CC init barrier + first
        # AllGathers finish ~60-100us in, so proj(0) at tb=1 would stall
        # the PE pipeline on the collective.
        for tb in range(cfg.TB):
            stage1(tb)
            attention(tb)
            if tb > 1:
                proj(tb - 2)
        proj(cfg.TB - 2)
        proj(cfg.TB - 1)


def make_core_inputs(x_full, c_attn_w, c_attn_b, c_proj_w, c_proj_b, cfg,
                     core):
    """Host-side input sharding for one core."""
    GS, HL, NP, T = cfg.GS, cfg.HL, cfg.NP, cfg.T
    g, rk = divmod(core, GS)
    g = g % B  # tolerate more groups than batches (sim configs)
    hs = slice(rk * HL * D, (rk + 1) * HL * D)
    wq = c_attn_w[:, 0 * C:1 * C][:, hs]
    wk = c_attn_w[:, 1 * C:2 * C][:, hs]
    wv = c_attn_w[:, 2 * C:3 * C][:, hs]
    bq = c_attn_b[0 * C:1 * C][hs]
    bk = c_attn_b[1 * C:2 * C][hs]
    bv = c_attn_b[2 * C:3 * C][hs]
    cs = slice(rk * NP, (rk + 1) * NP)

    # c_proj rows permuted to match the gathered layout: half-major,
    # then rank-major, then 2 heads x 64 dims.
    perm = []
    for half in range(HL // 2):
        for r in range(GS):
            base = (r * HL + 2 * half) * D
            perm.extend(range(base, base + 2 * D))
    wp = c_proj_w[np.array(perm)][:, cs]

    pp = np.arange(P)[:, None, None]
    jj = np.arange(4)[None, :, None]
    qq = np.arange(512)[None, None, :]
    masks = (qq >= pp + 128 * jj).astype(NP_BF16)

    bf = NP_BF16
    return {
        "x": np.ascontiguousarray(x_full[g, :T]).astype(bf),
        "wqk": np.ascontiguousarray(
            np.concatenate([wq, wk], axis=1)).astype(bf),
        "wv": np.ascontiguousarray(wv).astype(bf),
        "bqk": np.ascontiguousarray(
            np.concatenate([bq, bk]).reshape(cfg.QKCH, P).T, np.float32),
        "bv": np.ascontiguousarray(bv[None, :]).astype(bf),
        "wp": np.ascontiguousarray(wp).astype(bf),
        "bp": np.ascontiguousarray(c_proj_b[cs][None, :]).astype(bf),
        "masks": masks,
        "ident": np.eye(P).astype(bf),
    }


_CACHE = {}


def _build_full():
    if "nc" in _CACHE:
        return _CACHE["nc"]
    cfg = CFG_FULL
    nc = bacc.Bacc(
        "TRN2", target_bir_lowering=False, debug=False,
        num_devices=cfg.n_cores,
    )
    ins = {}
    shapes = {
        "x": ((cfg.T, C), BF16),
        "wqk": ((C, cfg.QKCH * P), BF16),
        "wv": ((C, cfg.HL * D), BF16),
        "bqk": ((P, cfg.QKCH), F32),
        "bv": ((1, cfg.HL * D), BF16),
        "wp": ((C, cfg.NP), BF16),
        "bp": ((1, cfg.NP), BF16),
        "masks": ((P, 4, 512), BF16),
        "ident": ((P, P), BF16),
    }
    for name, (shape, dt) in shapes.items():
        ins[name] = nc.dram_tensor(
            name, list(shape), dt, kind="ExternalInput").ap()
    outs = {
        "out": nc.dram_tensor(
            "out", [cfg.T, cfg.NP], F32, kind="ExternalOutput").ap()
    }
    with tile.TileContext(nc) as tc:
        emit(tc, outs, ins, cfg)
    nc.compile()
    _CACHE["nc"] = nc
    return nc


def kernel(**inputs):
    from concourse.bass_utils import run_bass_kernel_spmd

    cfg = CFG_FULL
    x = np.asarray(inputs["x"], np.float32)
    c_attn_w = np.asarray(inputs["c_attn_w"], np.float32)
    c_attn_b = np.asarray(inputs["c_attn_b"], np.float32)
    c_proj_w = np.asarray(inputs["c_proj_w"], np.float32)
    c_proj_b = np.asarray(inputs["c_proj_b"], np.float32)

    nc = _build_full()
    in_maps = [
        make_core_inputs(x, c_attn_w, c_attn_b, c_proj_w, c_proj_b, cfg, core)
        for core in range(cfg.n_cores)
    ]
    res = run_bass_kernel_spmd(nc, in_maps, core_ids=list(range(cfg.n_cores)))
    out = np.empty((B, T_FULL, C), np.float32)
    for core in range(cfg.n_cores):
        g, rk = divmod(core, cfg.GS)
        out[g, :, rk * cfg.NP:(rk + 1) * cfg.NP] = res.results[core]["out"]
    return out


# revision 29
# speedup vs baseline: 1.1401x; 1.0385x over previous
"""GPT-2 style multi-head attention on 8 Trainium2 cores (Bass/Tile).

Problem: B=2, T=2048, C=1024, H=16 heads, D=64, fp32 in/out.

Sharding (hardcoded): 2 groups x 4 cores; group g handles batch b=g.
Within a group, rank r computes heads [4r, 4r+4) (tensor parallel over
heads: c_attn column slices), then AllGather of y^T across the group,
then each core computes a 256-column slice of the output projection
(c_proj column slice) plus bias.

Datapath is bf16 (inputs converted on host): all matmul operands are
bf16 with fp32 PSUM accumulation, DVE elementwise runs at 2x on 16-bit,
and weight/x/collective DMA bytes are halved vs fp32.  The softmax
denominator reciprocal + broadcast stays fp32/f32r.  Measured rel err vs
the fp32 reference ~4e-3, well inside the 2e-2 gate.

Kernel dataflow per core, fully pipelined over 512-row t-blocks:
  per t-block tb:
    stage 1: x[tb] -> (PE transpose) x^T;  qk^T[.,tb] = W_qk^T @ x^T
             (per-partition f32 bias on PSUM copyback); V[tb] = x @ W_v
             (bias via ones-row matmul into the accumulating PSUM).  V is
             stored per 128-row k-tile with an appended ones column so
             the AV matmul also emits the softmax denominator for free.
    stage 2 (q block qb=tb): per head pair (even/odd heads share qkT
             chunks on partition bases 0/64, so their QK matmuls run on
             disjoint PE row strips): scores^T[k,q] = K^T.T @ Q^T in
             PSUM -> exp(0.125*s) on ACT -> causal mask multiply
             (diagonal tiles only) -> y_aug^T[65,512] += V_aug^T @ exp^T
             over k tiles (row 64 = sum of exp).  Normalize: reciprocal
             of row 64, ones-matmul broadcast, multiply -> bf16 y slice.
    stage 3: per head pair, AllGather the [128,512] y^T slice across the
             4-core group (two half-size collectives per t-block, so the
             second one of block tb overlaps block tb+1's compute), then
             out[:, col slice] = y^T.T @ W_p slice + bias for the
             previous t-block.  W_p rows are permuted on the host so the
             two gathered halves are contiguous contraction chunks.
"""

import numpy as np
import ml_dtypes

import concourse.bass as bass
import concourse.mybir as mybir
import concourse.tile as tile
from concourse import bacc

P = 128
B, T_FULL, C, H, D = 2, 2048, 1024, 16, 64
F32 = mybir.dt.float32
F32R = mybir.dt.float32r
BF16 = mybir.dt.bfloat16
NP_BF16 = ml_dtypes.bfloat16
EXP = mybir.ActivationFunctionType.Exp
LN = mybir.ActivationFunctionType.Ln
ADD = mybir.AluOpType.add
MUL = mybir.AluOpType.mult
BYPASS = mybir.AluOpType.bypass


class Cfg:
    def __init__(self, n_cores, group_size, T, fake_collective=False,
                 repeat=1, xt_bufs=3, e_bufs=8, x_bufs=4, n_bufs=3):
        self.fake_collective = fake_collective
        self.repeat = repeat
        self.xt_bufs = xt_bufs
        self.e_bufs = e_bufs
        self.x_bufs = x_bufs
        self.n_bufs = n_bufs
        self.n_cores = n_cores
        self.GS = group_size               # cores per batch group
        self.T = T                         # sequence length handled per core
        self.HL = H // group_size          # heads per core
        assert self.HL % 2 == 0
        self.NP = C // group_size          # output-projection columns per core
        self.CC = C // P                   # contraction chunks (8)
        self.TB = T // 512                 # t-blocks == q blocks
        self.QB = T // 512
        self.KT = T // P                   # k tiles
        self.QKCH = self.HL                # qk^T partition chunks (Q | K)
        self.VW = 68                       # per-head V stride: 64 V + 1 ones
        self.HP = self.HL // 2             # head pairs == AG halves per block
        if n_cores == 8:
            self.replica_groups = [[0, 1, 2, 3], [4, 5, 6, 7]]
        elif n_cores == 4:
            self.replica_groups = [[0, 1], [2, 3]]
        elif n_cores == 1:
            self.replica_groups = [[0]]
        else:
            raise ValueError(n_cores)


CFG_FULL = Cfg(8, 4, T_FULL)


def emit(tc, outs, ins, cfg):
    """Emit the SPMD program. outs/ins are dicts of DRAM APs."""
    for rep in range(cfg.repeat):
        _emit_once(tc, outs["out"], ins, cfg, rep)


def _emit_once(tc, out, ins, cfg, rep):
    nc = tc.nc
    GS, T, HL, NP, CC, VW = cfg.GS, cfg.T, cfg.HL, cfg.NP, cfg.CC, cfg.VW
    QKCH = cfg.QKCH

    xt = ins["xt"]            # [TB, P, CC, 512] bf16: host-transposed x
    wqk = ins["wqk"]          # [P, CC, HL*128] bf16  (Q cols | K cols)
    wv = ins["wv"]            # [P, CC, HL*64] bf16
    bqk = ins["bqk"]          # [P, HL] f32  (chunk-major per-partition bias)
    bv = ins["bv"]            # [1, HL*64] bf16
    wp = ins["wp"]            # [P, CC, NP] bf16 (rows permuted: AG halves)
    bp = ins["bp"]            # [1, NP] bf16
    masks = ins["masks"]      # [P, 4, 512] bf16

    from contextlib import ExitStack
    with ExitStack() as _stk:
        persist = _stk.enter_context(tc.tile_pool(name="persist", bufs=1))
        s1 = _stk.enter_context(tc.tile_pool(name="s1", bufs=2))
        s2 = _stk.enter_context(tc.tile_pool(name="s2", bufs=4))
        s3 = _stk.enter_context(tc.tile_pool(name="s3", bufs=2))
        dram = _stk.enter_context(
            tc.tile_pool(name="dram", bufs=1, space="DRAM"))
        ps_acc = _stk.enter_context(tc.tile_pool(
            name="ps_acc", bufs=2, space="PSUM"))
        ps_s = _stk.enter_context(tc.tile_pool(
            name="ps_s", bufs=3, space="PSUM"))
        ps_y = _stk.enter_context(tc.tile_pool(
            name="ps_y", bufs=3, space="PSUM"))
        # ---- persistent SBUF tensors ----
        qkT = persist.tile([P, QKCH, T], BF16, tag="qkT")

        def qk_write(m, tb):
            return qkT[:, m, tb * 512:(tb + 1) * 512]

        def qk_q(pb, qch, qb, lo):
            return qkT[pb, qch, qb * 512 + lo:(qb + 1) * 512]

        def qk_k(pb, kch, kt):
            return qkT[pb, kch, kt * P:(kt + 1) * P]

        vsb = persist.tile([P, cfg.KT, HL * VW], BF16, tag="vsb")
        mask_sb = persist.tile([P, 4, 512], BF16, tag="mask")
        ones_row = persist.tile([1, P], BF16, tag="ones_row")
        wp_sb = persist.tile([P, CC, NP], BF16, tag="wp")
        bp_sb = persist.tile([1, NP], BF16, tag="bp")
        wqk_sb = persist.tile([P, CC, QKCH * P], BF16, tag="wqk")
        wv_sb = persist.tile([P, CC, HL * D], BF16, tag="wv")
        bqk_sb = persist.tile([P, QKCH], F32, tag="bqk")
        bv_sb = persist.tile([1, HL * D], BF16, tag="bv")

        # x arrives pre-transposed and tb-sliced from the host:
        # xt[tb, p, cc, u] with 8 KB contiguous per partition per slab.
        # Weights are host-prepacked to the SBUF layout [p, cc, m] so each
        # load is one descriptor with large contiguous packets.
        # Queues: sync = x slabs (+ yn / ag_in writes); scalar = wqk,
        # masks, later ag/out traffic; gpsimd = the rest + AG triggers.
        nc.scalar.dma_start(wqk_sb[:], wqk)
        nc.gpsimd.dma_start(wv_sb[:], wv)
        nc.gpsimd.dma_start(bqk_sb[:], bqk)
        nc.gpsimd.dma_start(bv_sb[:], bv)
        nc.scalar.dma_start(mask_sb[:], masks)
        nc.gpsimd.dma_start(wp_sb[:], wp)
        nc.gpsimd.dma_start(bp_sb[:], bp)

        # memset can't write f32r/bf16; memset f32 scratch, copy-convert.
        scratch1 = persist.tile([P, max(P, cfg.KT * HL)], F32, tag="scratch1")
        nc.vector.memset(scratch1[:], 1.0)
        nc.vector.tensor_copy(ones_row[:], scratch1[0:1, 0:P])
        # ones columns inside the V tile (col 64 of each head's 68-wide slot)
        vsb_h = vsb.rearrange("p k (h w) -> p k h w", w=VW)
        nc.vector.tensor_copy(
            vsb_h[:, :, :, 64:65],
            scratch1[:, 0:cfg.KT * HL].rearrange(
                "p (k h o) -> p k h o", k=cfg.KT, h=HL, o=1),
        )

        # per (t-block, head-pair) AllGather buffers: in [128, 512] out
        # [GS*128, 512], both bf16.
        ag_in = [
            [dram.tile([2 * D, 512], BF16, tag=f"agin{qb}_{hp}",
                       name=f"agin{qb}_{hp}_{rep}")
             for hp in range(cfg.HP)]
            for qb in range(cfg.QB)
        ]
        ag_out = [
            [dram.tile([GS * 2 * D, 512], BF16, tag=f"agout{qb}_{hp}",
                       name=f"agout{qb}_{hp}_{rep}")
             for hp in range(cfg.HP)]
            for qb in range(cfg.QB)
        ]

        def stage1(tb):
            xT = s1.tile([P, CC, 512], BF16, tag="xT", bufs=cfg.xt_bufs)
            nc.sync.dma_start(xT[:], xt[tb])
            # qk^T: lhsT = W chunk, rhs = x^T chunk
            for m in range(QKCH):
                acc = ps_acc.tile([P, 512], F32, tag="acc")
                for cc in range(CC):
                    nc.tensor.matmul(
                        acc[:],
                        wqk_sb[:, cc, m * P:(m + 1) * P],
                        xT[:, cc, :],
                        start=(cc == 0),
                        stop=(cc == CC - 1),
                    )
                nc.vector.tensor_scalar_add(
                    qk_write(m, tb), acc[:],
                    bqk_sb[:, m:m + 1],
                )
            # V natural: lhsT = x^T chunk, rhs = W_v
            for ts in range(4):
                kt = tb * 4 + ts
                vp = ps_acc.tile([P, 512], F32, tag="acc")
                for cc in range(CC):
                    nc.tensor.matmul(
                        vp[:, 0:HL * D],
                        xT[:, cc, ts * P:(ts + 1) * P],
                        wv_sb[:, cc, :],
                        start=(cc == 0),
                        stop=False,
                    )
                nc.tensor.matmul(
                    vp[:, 0:HL * D], ones_row[:1, :], bv_sb[:1, :],
                    start=False, stop=True,
                )
                nc.vector.tensor_copy(
                    vsb_h[:, kt, :, 0:64],
                    vp[:, 0:HL * D].rearrange("p (h d) -> p h d", d=D),
                )

        def attention(qb):
            # even/odd head pairs sit on partition bases 0 and 64 of the
            # same qkT chunks; interleaving their QK matmuls puts them on
            # disjoint PE row strips (tile_position auto-derived), so the
            # two 64-contract matmuls run concurrently in the array.
            nkt = 4 * qb + 4
            kt_order = list(range(4 * qb, nkt)) + list(range(0, 4 * qb))
            for hp in range(cfg.HP):
                hs = (2 * hp, 2 * hp + 1)
                qch, kch = hp, QKCH // 2 + hp
                pbs = [slice((h % 2) * 64, (h % 2) * 64 + 64) for h in hs]
                ys = [ps_y.tile([65, 512], F32, tag="y",
                                name=f"y{qb}_{h}") for h in hs]
                for ki, kt in enumerate(kt_order):
                    j = kt - 4 * qb
                    lo = 128 * j if j > 0 else 0
                    ss, es = [], []
                    for i in range(2):
                        s = ps_s.tile([P, 512], F32, tag="s",
                                      name=f"s{qb}_{kt}_{i}")
                        nc.tensor.matmul(
                            s[:, lo:],
                            qk_k(pbs[i], kch, kt),
                            qk_q(pbs[i], qch, qb, lo),
                            start=True, stop=True,
                        )
                        ss.append(s)
                    for i in range(2):
                        e = s2.tile([P, 512], BF16, tag="e",
                                    bufs=cfg.e_bufs,
                                    name=f"e{qb}_{kt}_{i}")
                        nc.scalar.activation(
                            e[:, lo:], ss[i][:, lo:], EXP, scale=0.125)
                        if j >= 0:
                            nc.vector.tensor_mul(
                                e[:, lo:], e[:, lo:], mask_sb[:, j, lo:])
                        es.append(e)
                    for i in range(2):
                        nc.tensor.matmul(
                            ys[i][:, lo:],
                            vsb[:, kt, hs[i] * VW:hs[i] * VW + 65],
                            es[i][:, lo:],
                            start=(ki == 0), stop=(ki == nkt - 1),
                        )
                for i in range(2):
                    _normalize(qb, hp, i, ys[i])
                allgather(qb, hp)

        def _normalize(qb, hp, i, y):
            # 1/den via the fast-approx custom DVE op (~18 bits, one op):
            # the exact DVE reciprocal is single-lane ~3.3us on [1,512],
            # and the Ln/Exp ACT alternative forces ~1.3us activation
            # table reloads between it and the scores exps.
            h = 2 * hp + i
            den = s2.tile([1, 512], F32, tag="den", bufs=cfg.n_bufs,
                          name=f"den{qb}_{h}")
            nc.vector.tensor_copy(den[:], y[64:65, :])
            rec = s2.tile([1, 512], F32, tag="rec", bufs=cfg.n_bufs,
                          name=f"rec{qb}_{h}")
            nc.vector.reciprocal_approx_fast(rec[:], den[:])
            rec_bf = s2.tile([1, 512], BF16, tag="rec_bf", bufs=cfg.n_bufs,
                             name=f"recb{qb}_{h}")
            nc.vector.tensor_copy(rec_bf[:], rec[:])
            bc = ps_s.tile([P, 512], F32, tag="s", name=f"bc{qb}_{h}")
            nc.tensor.matmul(
                bc[0:64, :], ones_row[:1, 0:64], rec_bf[:1, :],
                start=True, stop=True,
            )
            bc_sb = s2.tile([64, 512], F32, tag="bc_sb", bufs=cfg.n_bufs,
                            name=f"bcs{qb}_{h}")
            nc.vector.tensor_copy(bc_sb[:], bc[0:64, :])
            yn = s2.tile([64, 512], BF16, tag="yn", bufs=cfg.n_bufs,
                         name=f"yn{qb}_{h}")
            nc.vector.tensor_mul(yn[:], y[0:64, :], bc_sb[:])
            nc.sync.dma_start(ag_in[qb][hp][i * 64:(i + 1) * 64, :], yn[:])

        def allgather(qb, hp):
            if cfg.fake_collective:
                # timing-model variant (TimelineSim can't run collectives):
                # stand-in DRAM->DRAM copy.
                nc.sync.dma_start(
                    ag_out[qb][hp][0:2 * D, :], ag_in[qb][hp][:])
                return
            nc.gpsimd.collective_compute(
                "AllGather", BYPASS,
                replica_groups=cfg.replica_groups,
                ins=[ag_in[qb][hp].opt()],
                outs=[ag_out[qb][hp].opt()],
            )

        def proj(qb):
            # contraction rows: half 0 = ranks x heads {0,1}, half 1 =
            # ranks x heads {2,3}; wp rows are host-permuted to match.
            # One bulk DMA per gathered half (512 KB streams at full rate)
            # instead of per-t-block strided loads.
            ag_sb = [
                s3.tile([P, CC // 2, 512], BF16, tag=f"ag{hp}",
                        name=f"ag{qb}_{hp}")
                for hp in range(cfg.HP)
            ]
            # scalar queue: an ag load waits on its AllGather semaphore,
            # and on the sync queue that would head-of-line block the
            # x-tile and yn DMAs of later blocks.
            for hp in range(cfg.HP):
                nc.scalar.dma_start(
                    ag_sb[hp][:],
                    ag_out[qb][hp].rearrange("(c p) t -> p c t", p=P))
            for tt in range(4):
                op = ps_acc.tile([P, 512], F32, tag="acc")
                for cc in range(CC):
                    hp, c = divmod(cc, CC // 2)
                    nc.tensor.matmul(
                        op[:, 0:NP],
                        ag_sb[hp][:, c, tt * P:(tt + 1) * P],
                        wp_sb[:, cc, :], start=(cc == 0), stop=False,
                    )
                nc.tensor.matmul(
                    op[:, 0:NP], ones_row[:1, :], bp_sb[:1, :],
                    start=False, stop=True,
                )
                o_sb = s3.tile([P, NP], F32, tag="osb")
                nc.vector.tensor_copy(o_sb[:], op[:, 0:NP])
                row = (qb * 4 + tt) * P
                nc.scalar.dma_start(out[row:row + P, :], o_sb[:])

        # fused pipeline: attention(qb) needs exactly the k-tiles stage1(tb)
        # has produced; the AllGathers fire per head pair inside
        # attention(), so the later ones overlap the next block's compute.
        # proj is deferred by TWO blocks: the CC init barrier + first
        # AllGathers finish ~60-100us in, so proj(0) at tb=1 would stall
        # the PE pipeline on the collective.
        for tb in range(cfg.TB):
            stage1(tb)
            attention(tb)
            if tb > 1:
                proj(tb - 2)
        proj(cfg.TB - 2)
        proj(cfg.TB - 1)


def make_core_inputs(x_full, c_attn_w, c_attn_b, c_proj_w, c_proj_b, cfg,
                     core):
    """Host-side input sharding for one core."""
    GS, HL, NP, T = cfg.GS, cfg.HL, cfg.NP, cfg.T
    g, rk = divmod(core, GS)
    g = g % B  # tolerate more groups than batches (sim configs)
    hs = slice(rk * HL * D, (rk + 1) * HL * D)
    wq = c_attn_w[:, 0 * C:1 * C][:, hs]
    wk = c_attn_w[:, 1 * C:2 * C][:, hs]
    wv = c_attn_w[:, 2 * C:3 * C][:, hs]
    bq = c_attn_b[0 * C:1 * C][hs]
    bk = c_attn_b[1 * C:2 * C][hs]
    bv = c_attn_b[2 * C:3 * C][hs]
    cs = slice(rk * NP, (rk + 1) * NP)

    # c_proj rows permuted to match the gathered layout: half-major,
    # then rank-major, then 2 heads x 64 dims.
    perm = []
    for half in range(HL // 2):
        for r in range(GS):
            base = (r * HL + 2 * half) * D
            perm.extend(range(base, base + 2 * D))
    wp = c_proj_w[np.array(perm)][:, cs]

    pp = np.arange(P)[:, None, None]
    jj = np.arange(4)[None, :, None]
    qq = np.arange(512)[None, None, :]
    masks = (qq >= pp + 128 * jj).astype(NP_BF16)

    bf = NP_BF16
    CC = cfg.CC

    def pack(w):
        # [C, M] -> SBUF layout [P, CC, M]: partition-contiguous packets.
        m = w.shape[1]
        return np.ascontiguousarray(
            w.reshape(CC, P, m).transpose(1, 0, 2)).astype(bf)

    xT = x_full[g, :T].astype(bf).T        # [C, T] in bf16
    xt = np.ascontiguousarray(
        xT.reshape(CC, P, cfg.TB, 512).transpose(2, 1, 0, 3))

    return {
        "xt": xt,
        "wqk": pack(np.concatenate([wq, wk], axis=1)),
        "wv": pack(wv),
        "bqk": np.ascontiguousarray(
            np.concatenate([bq, bk]).reshape(cfg.QKCH, P).T, np.float32),
        "bv": np.ascontiguousarray(bv[None, :]).astype(bf),
        "wp": pack(wp),
        "bp": np.ascontiguousarray(c_proj_b[cs][None, :]).astype(bf),
        "masks": masks,
    }


_CACHE = {}


def _build_full():
    if "nc" in _CACHE:
        return _CACHE["nc"]
    cfg = CFG_FULL
    nc = bacc.Bacc(
        "TRN2", target_bir_lowering=False, debug=False,
        num_devices=cfg.n_cores,
    )
    ins = {}
    shapes = {
        "xt": ((cfg.TB, P, cfg.CC, 512), BF16),
        "wqk": ((P, cfg.CC, cfg.QKCH * P), BF16),
        "wv": ((P, cfg.CC, cfg.HL * D), BF16),
        "bqk": ((P, cfg.QKCH), F32),
        "bv": ((1, cfg.HL * D), BF16),
        "wp": ((P, cfg.CC, cfg.NP), BF16),
        "bp": ((1, cfg.NP), BF16),
        "masks": ((P, 4, 512), BF16),
    }
    for name, (shape, dt) in shapes.items():
        ins[name] = nc.dram_tensor(
            name, list(shape), dt, kind="ExternalInput").ap()
    outs = {
        "out": nc.dram_tensor(
            "out", [cfg.T, cfg.NP], F32, kind="ExternalOutput").ap()
    }
    with tile.TileContext(nc) as tc:
        emit(tc, outs, ins, cfg)
    nc.compile()
    _CACHE["nc"] = nc
    return nc


def kernel(**inputs):
    from concourse.bass_utils import run_bass_kernel_spmd

    cfg = CFG_FULL
    x = np.asarray(inputs["x"], np.float32)
    c_attn_w = np.asarray(inputs["c_attn_w"], np.float32)
    c_attn_b = np.asarray(inputs["c_attn_b"], np.float32)
    c_proj_w = np.asarray(inputs["c_proj_w"], np.float32)
    c_proj_b = np.asarray(inputs["c_proj_b"], np.float32)

    nc = _build_full()
    in_maps = [
        make_core_inputs(x, c_attn_w, c_attn_b, c_proj_w, c_proj_b, cfg, core)
        for core in range(cfg.n_cores)
    ]
    res = run_bass_kernel_spmd(nc, in_maps, core_ids=list(range(cfg.n_cores)))
    out = np.empty((B, T_FULL, C), np.float32)
    for core in range(cfg.n_cores):
        g, rk = divmod(core, cfg.GS)
        out[g, :, rk * cfg.NP:(rk + 1) * cfg.NP] = res.results[core]["out"]
    return out


# revision 37
# speedup vs baseline: 1.2959x; 1.1367x over previous
"""GPT-2 style multi-head attention on 8 Trainium2 cores (Bass/Tile).

Problem: B=2, T=2048, C=1024, H=16 heads, D=64, fp32 in/out.

Sharding (hardcoded): 2 groups x 4 cores; group g handles batch b=g.
Within a group, rank r computes heads [4r, 4r+4) (tensor parallel over
heads: c_attn column slices), then AllGather of y^T across the group,
then each core computes a 256-column slice of the output projection
(c_proj column slice) plus bias.

Datapath is bf16 (inputs converted on host): all matmul operands are
bf16 with fp32 PSUM accumulation, DVE elementwise runs at 2x on 16-bit,
and weight/x/collective DMA bytes are halved vs fp32.  The softmax
denominator reciprocal + broadcast stays fp32/f32r.  Measured rel err vs
the fp32 reference ~4e-3, well inside the 2e-2 gate.

Kernel dataflow per core, fully pipelined over 512-row t-blocks:
  per t-block tb:
    stage 1: x[tb] -> (PE transpose) x^T;  qk^T[.,tb] = W_qk^T @ x^T
             (per-partition f32 bias on PSUM copyback); V[tb] = x @ W_v
             (bias via ones-row matmul into the accumulating PSUM).  V is
             stored per 128-row k-tile with an appended ones column so
             the AV matmul also emits the softmax denominator for free.
    stage 2 (q block qb=tb): per head pair (even/odd heads share qkT
             chunks on partition bases 0/64, so their QK matmuls run on
             disjoint PE row strips): scores^T[k,q] = K^T.T @ Q^T in
             PSUM -> exp(0.125*s) on ACT -> causal mask multiply
             (diagonal tiles only) -> y_aug^T[65,512] += V_aug^T @ exp^T
             over k tiles (row 64 = sum of exp).  Normalize: reciprocal
             of row 64, ones-matmul broadcast, multiply -> bf16 y slice.
    stage 3: per head pair, AllGather the [128,512] y^T slice across the
             4-core group (two half-size collectives per t-block, so the
             second one of block tb overlaps block tb+1's compute), then
             out[:, col slice] = y^T.T @ W_p slice + bias for the
             previous t-block.  W_p rows are permuted on the host so the
             two gathered halves are contiguous contraction chunks.
"""

import numpy as np
import ml_dtypes

import concourse.bass as bass
import concourse.mybir as mybir
import concourse.tile as tile
from concourse import bacc

P = 128
B, T_FULL, C, H, D = 2, 2048, 1024, 16, 64
F32 = mybir.dt.float32
F32R = mybir.dt.float32r
BF16 = mybir.dt.bfloat16
NP_BF16 = ml_dtypes.bfloat16
EXP = mybir.ActivationFunctionType.Exp
LN = mybir.ActivationFunctionType.Ln
ADD = mybir.AluOpType.add
MUL = mybir.AluOpType.mult
BYPASS = mybir.AluOpType.bypass


class Cfg:
    def __init__(self, n_cores, group_size, T, fake_collective=False,
                 repeat=1, xt_bufs=3, e_bufs=8, x_bufs=4, n_bufs=3):
        self.fake_collective = fake_collective
        self.repeat = repeat
        self.xt_bufs = xt_bufs
        self.e_bufs = e_bufs
        self.x_bufs = x_bufs
        self.n_bufs = n_bufs
        self.n_cores = n_cores
        self.GS = group_size               # cores per batch group
        self.T = T                         # sequence length handled per core
        self.HL = H // group_size          # heads per core
        assert self.HL % 2 == 0
        self.NP = C // group_size          # output-projection columns per core
        self.CC = C // P                   # contraction chunks (8)
        self.TB = T // 512                 # t-blocks == q blocks
        self.QB = T // 512
        self.KT = T // P                   # k tiles
        self.QKCH = self.HL                # qk^T partition chunks (Q | K)
        self.VW = 68                       # per-head V stride: 64 V + 1 ones
        self.HP = self.HL // 2             # head pairs == AG halves per block
        if n_cores == 8:
            self.replica_groups = [[0, 1, 2, 3], [4, 5, 6, 7]]
        elif n_cores == 4:
            self.replica_groups = [[0, 1], [2, 3]]
        elif n_cores == 1:
            self.replica_groups = [[0]]
        else:
            raise ValueError(n_cores)


CFG_FULL = Cfg(8, 4, T_FULL)


def emit(tc, outs, ins, cfg):
    """Emit the SPMD program. outs/ins are dicts of DRAM APs."""
    for rep in range(cfg.repeat):
        _emit_once(tc, outs["out"], ins, cfg, rep)


def _emit_once(tc, out, ins, cfg, rep):
    nc = tc.nc
    GS, T, HL, NP, CC, VW = cfg.GS, cfg.T, cfg.HL, cfg.NP, cfg.CC, cfg.VW
    QKCH = cfg.QKCH

    xt = ins["xt"]            # [TB, P, CC, 512] bf16: host-transposed x
    wqk = ins["wqk"]          # [P, CC, HL*128] bf16  (Q cols | K cols)
    wv = ins["wv"]            # [P, CC, HL*64] bf16
    bqk = ins["bqk"]          # [P, HL] f32  (chunk-major per-partition bias)
    bv = ins["bv"]            # [1, HL*64] bf16
    wp = ins["wp"]            # [P, CC, NP] bf16 (rows permuted: AG halves)
    bp = ins["bp"]            # [1, NP] bf16
    masks = ins["masks"]      # [P, 4, 512] bf16

    from contextlib import ExitStack
    with ExitStack() as _stk:
        persist = _stk.enter_context(tc.tile_pool(name="persist", bufs=1))
        s1 = _stk.enter_context(tc.tile_pool(name="s1", bufs=2))
        s2 = _stk.enter_context(tc.tile_pool(name="s2", bufs=4))
        s3 = _stk.enter_context(tc.tile_pool(name="s3", bufs=2))
        dram = _stk.enter_context(
            tc.tile_pool(name="dram", bufs=1, space="DRAM"))
        # 8 PSUM banks: acc 1 + bc 1 + s (wide, 2 banks each) 4 + y 2
        ps_acc = _stk.enter_context(tc.tile_pool(
            name="ps_acc", bufs=1, space="PSUM"))
        ps_s = _stk.enter_context(tc.tile_pool(
            name="ps_s", bufs=2, space="PSUM"))
        ps_y = _stk.enter_context(tc.tile_pool(
            name="ps_y", bufs=2, space="PSUM"))
        # ---- persistent SBUF tensors ----
        qkT = persist.tile([P, QKCH, T], BF16, tag="qkT")

        def qk_write(m, tb):
            return qkT[:, m, tb * 512:(tb + 1) * 512]

        def qk_q(pb, qch, qb, lo):
            return qkT[pb, qch, qb * 512 + lo:(qb + 1) * 512]

        def qk_k(pb, kch, kt):
            return qkT[pb, kch, kt * P:(kt + 1) * P]

        vsb = persist.tile([P, cfg.KT, HL * VW], BF16, tag="vsb")
        mask_sb = persist.tile([P, 4, 512], BF16, tag="mask")
        ones_row = persist.tile([1, P], BF16, tag="ones_row")
        ones65 = persist.tile([65, 64], BF16, tag="ones65")
        wp_sb = persist.tile([P, CC, NP], BF16, tag="wp")
        bp_sb = persist.tile([1, NP], BF16, tag="bp")
        wqk_sb = persist.tile([P, CC, QKCH * P], BF16, tag="wqk")
        wv_sb = persist.tile([P, CC, HL * D], BF16, tag="wv")
        bqk_sb = persist.tile([P, QKCH], F32, tag="bqk")
        bv_sb = persist.tile([1, HL * D], BF16, tag="bv")

        # x arrives pre-transposed and tb-sliced from the host:
        # xt[tb, p, cc, u] with 8 KB contiguous per partition per slab.
        # Weights are host-prepacked to the SBUF layout [p, cc, m] so each
        # load is one descriptor with large contiguous packets.
        # Queues: sync = x slabs (+ yn / ag_in writes); scalar = wqk,
        # masks, later ag/out traffic; gpsimd = the rest + AG triggers.
        # wqk on the sync ring ahead of the xt slabs: measured ~350 GB/s
        # vs ~75 GB/s on the scalar ring; the first qk matmul needs it.
        nc.sync.dma_start(wqk_sb[:], wqk)
        nc.gpsimd.dma_start(wv_sb[:], wv)
        nc.gpsimd.dma_start(bqk_sb[:], bqk)
        nc.gpsimd.dma_start(bv_sb[:], bv)
        nc.scalar.dma_start(mask_sb[:], masks)
        nc.gpsimd.dma_start(wp_sb[:], wp)
        nc.gpsimd.dma_start(bp_sb[:], bp)

        # memset can't write f32r/bf16; memset f32 scratch, copy-convert.
        scratch1 = persist.tile([P, max(P, cfg.KT * HL)], F32, tag="scratch1")
        nc.vector.memset(scratch1[:], 1.0)
        nc.vector.tensor_copy(ones_row[:], scratch1[0:1, 0:P])
        nc.vector.tensor_copy(ones65[0:1, :], scratch1[0:1, 0:64])
        nc.vector.tensor_copy(ones65[64:65, :], scratch1[64:65, 0:64])
        # ones columns inside the V tile (col 64 of each head's 68-wide slot)
        vsb_h = vsb.rearrange("p k (h w) -> p k h w", w=VW)
        nc.vector.tensor_copy(
            vsb_h[:, :, :, 64:65],
            scratch1[:, 0:cfg.KT * HL].rearrange(
                "p (k h o) -> p k h o", k=cfg.KT, h=HL, o=1),
        )

        # per (t-block, head-pair) AllGather buffers: in [128, 512] out
        # [GS*128, 512], both bf16.
        ag_in = [
            [dram.tile([2 * D, 512], BF16, tag=f"agin{qb}_{hp}",
                       name=f"agin{qb}_{hp}_{rep}")
             for hp in range(cfg.HP)]
            for qb in range(cfg.QB)
        ]
        ag_out = [
            [dram.tile([GS * 2 * D, 512], BF16, tag=f"agout{qb}_{hp}",
                       name=f"agout{qb}_{hp}_{rep}")
             for hp in range(cfg.HP)]
            for qb in range(cfg.QB)
        ]

        def stage1(tb):
            xT = s1.tile([P, CC, 512], BF16, tag="xT", bufs=cfg.xt_bufs)
            nc.sync.dma_start(xT[:], xt[tb])
            # qk^T: lhsT = W chunk, rhs = x^T chunk
            for m in range(QKCH):
                acc = ps_acc.tile([P, 512], F32, tag="acc")
                for cc in range(CC):
                    nc.tensor.matmul(
                        acc[:],
                        wqk_sb[:, cc, m * P:(m + 1) * P],
                        xT[:, cc, :],
                        start=(cc == 0),
                        stop=(cc == CC - 1),
                    )
                nc.vector.tensor_scalar_add(
                    qk_write(m, tb), acc[:],
                    bqk_sb[:, m:m + 1],
                )
            # V natural: lhsT = x^T chunk, rhs = W_v
            for ts in range(4):
                kt = tb * 4 + ts
                vp = ps_acc.tile([P, 512], F32, tag="acc")
                for cc in range(CC):
                    nc.tensor.matmul(
                        vp[:, 0:HL * D],
                        xT[:, cc, ts * P:(ts + 1) * P],
                        wv_sb[:, cc, :],
                        start=(cc == 0),
                        stop=False,
                    )
                nc.tensor.matmul(
                    vp[:, 0:HL * D], ones_row[:1, :], bv_sb[:1, :],
                    start=False, stop=True,
                )
                nc.vector.tensor_copy(
                    vsb_h[:, kt, :, 0:64],
                    vp[:, 0:HL * D].rearrange("p (h d) -> p h d", d=D),
                )

        def attention(qb):
            # even/odd head pairs sit on partition bases 0 and 64 of the
            # same qkT chunks; interleaving their QK matmuls puts them on
            # disjoint PE row strips (tile_position auto-derived), so the
            # two 64-contract matmuls run concurrently in the array.
            # Both heads' scores go into one 2-bank PSUM tile so a single
            # exp and a single broadcast-masked multiply cover the pair
            # (the ACT engine's ~400ns/op fixed cost was pacing the PE).
            nkt = 4 * qb + 4
            kt_order = list(range(4 * qb, nkt)) + list(range(0, 4 * qb))
            for hp in range(cfg.HP):
                hs = (2 * hp, 2 * hp + 1)
                qch, kch = hp, QKCH // 2 + hp
                pbs = [slice((h % 2) * 64, (h % 2) * 64 + 64) for h in hs]
                ys = [ps_y.tile([65, 512], F32, tag="y",
                                name=f"y{qb}_{h}") for h in hs]
                for ki, kt in enumerate(kt_order):
                    j = kt - 4 * qb
                    lo = 128 * j if j > 0 else 0
                    s = ps_s.tile([P, 2, 512], F32, tag="s",
                                  name=f"s{qb}_{kt}")
                    for i in range(2):
                        nc.tensor.matmul(
                            s[:, i, lo:],
                            qk_k(pbs[i], kch, kt),
                            qk_q(pbs[i], qch, qb, lo),
                            start=True, stop=True,
                        )
                    e = s2.tile([P, 2, 512], BF16, tag="e",
                                bufs=cfg.e_bufs, name=f"e{qb}_{kt}")
                    nc.scalar.activation(
                        e[:, :, lo:], s[:, :, lo:], EXP, scale=0.125)
                    if j >= 0:
                        nc.vector.tensor_mul(
                            e[:, :, lo:], e[:, :, lo:],
                            mask_sb[:, j:j + 1, lo:].to_broadcast(
                                [P, 2, 512 - lo]))
                    for i in range(2):
                        nc.tensor.matmul(
                            ys[i][:, lo:],
                            vsb[:, kt, hs[i] * VW:hs[i] * VW + 65],
                            e[:, i, lo:],
                            start=(ki == 0), stop=(ki == nkt - 1),
                        )
                _normalize_pair(qb, hp, ys)
                allgather(qb, hp)

        def _normalize_pair(qb, hp, ys):
            # 1/den via the fast-approx custom DVE op (~18 bits): the
            # exact DVE reciprocal is single-lane ~3.3us on [1,512], and
            # the Ln/Exp ACT alternative forces ~1.3us activation table
            # reloads between it and the scores exps.  Both heads share
            # one reciprocal + one bf16 cast.
            # heads' denominator rows at partition bases 0 and 64 (DVE
            # ops only accept bases 0/32/64); rows 1..63 hold garbage the
            # reciprocal maps to undefined values that nothing reads.
            den = s2.tile([65, 512], F32, tag="den", bufs=cfg.n_bufs,
                          name=f"den{qb}_{hp}")
            for i in range(2):
                nc.vector.tensor_copy(
                    den[64 * i:64 * i + 1, :], ys[i][64:65, :])
            rec = s2.tile([65, 512], F32, tag="rec", bufs=cfg.n_bufs,
                          name=f"rec{qb}_{hp}")
            nc.vector.reciprocal_approx_fast(rec[:], den[:])
            rec_bf = s2.tile([65, 512], BF16, tag="rec_bf",
                             bufs=cfg.n_bufs, name=f"recb{qb}_{hp}")
            nc.vector.tensor_copy(rec_bf[:], rec[:])
            for i in range(2):
                h = 2 * hp + i
                bc = ps_acc.tile([P, 512], F32, tag="bc",
                                 name=f"bc{qb}_{h}")
                nc.tensor.matmul(
                    bc[0:64, :], ones65[64 * i:64 * i + 1, :],
                    rec_bf[64 * i:64 * i + 1, :],
                    start=True, stop=True,
                )
                bc_sb = s2.tile([64, 512], F32, tag="bc_sb",
                                bufs=cfg.n_bufs, name=f"bcs{qb}_{h}")
                nc.vector.tensor_copy(bc_sb[:], bc[0:64, :])
                yn = s2.tile([64, 512], BF16, tag="yn", bufs=cfg.n_bufs,
                             name=f"yn{qb}_{h}")
                nc.vector.tensor_mul(yn[:], ys[i][0:64, :], bc_sb[:])
                nc.sync.dma_start(
                    ag_in[qb][hp][i * 64:(i + 1) * 64, :], yn[:])

        def allgather(qb, hp):
            if cfg.fake_collective:
                # timing-model variant (TimelineSim can't run collectives):
                # stand-in DRAM->DRAM copy.
                nc.sync.dma_start(
                    ag_out[qb][hp][0:2 * D, :], ag_in[qb][hp][:])
                return
            nc.gpsimd.collective_compute(
                "AllGather", BYPASS,
                replica_groups=cfg.replica_groups,
                ins=[ag_in[qb][hp].opt()],
                outs=[ag_out[qb][hp].opt()],
            )

        def proj(qb):
            # contraction rows: half 0 = ranks x heads {0,1}, half 1 =
            # ranks x heads {2,3}; wp rows are host-permuted to match.
            # One bulk DMA per gathered half (512 KB streams at full rate)
            # instead of per-t-block strided loads.
            ag_sb = [
                s3.tile([P, CC // 2, 512], BF16, tag=f"ag{hp}",
                        name=f"ag{qb}_{hp}")
                for hp in range(cfg.HP)
            ]
            # scalar queue: an ag load waits on its AllGather semaphore,
            # and on the sync queue that would head-of-line block the
            # x-tile and yn DMAs of later blocks.
            for hp in range(cfg.HP):
                nc.scalar.dma_start(
                    ag_sb[hp][:],
                    ag_out[qb][hp].rearrange("(c p) t -> p c t", p=P))
            for tt in range(4):
                op = ps_acc.tile([P, 512], F32, tag="acc")
                for cc in range(CC):
                    hp, c = divmod(cc, CC // 2)
                    nc.tensor.matmul(
                        op[:, 0:NP],
                        ag_sb[hp][:, c, tt * P:(tt + 1) * P],
                        wp_sb[:, cc, :], start=(cc == 0), stop=False,
                    )
                nc.tensor.matmul(
                    op[:, 0:NP], ones_row[:1, :], bp_sb[:1, :],
                    start=False, stop=True,
                )
                o_sb = s3.tile([P, NP], F32, tag="osb")
                nc.vector.tensor_copy(o_sb[:], op[:, 0:NP])
                row = (qb * 4 + tt) * P
                nc.scalar.dma_start(out[row:row + P, :], o_sb[:])

        # fused pipeline: attention(qb) needs exactly the k-tiles stage1(tb)
        # has produced; the AllGathers fire per head pair inside
        # attention(), so the later ones overlap the next block's compute.
        # proj is deferred by TWO blocks: the CC init barrier + first
        # AllGathers finish ~60-100us in, so proj(0) at tb=1 would stall
        # the PE pipeline on the collective.
        for tb in range(cfg.TB):
            stage1(tb)
            attention(tb)
            if tb > 1:
                proj(tb - 2)
        proj(cfg.TB - 2)
        proj(cfg.TB - 1)


def make_core_inputs(x_full, c_attn_w, c_attn_b, c_proj_w, c_proj_b, cfg,
                     core):
    """Host-side input sharding for one core."""
    GS, HL, NP, T = cfg.GS, cfg.HL, cfg.NP, cfg.T
    g, rk = divmod(core, GS)
    g = g % B  # tolerate more groups than batches (sim configs)
    hs = slice(rk * HL * D, (rk + 1) * HL * D)
    wq = c_attn_w[:, 0 * C:1 * C][:, hs]
    wk = c_attn_w[:, 1 * C:2 * C][:, hs]
    wv = c_attn_w[:, 2 * C:3 * C][:, hs]
    bq = c_attn_b[0 * C:1 * C][hs]
    bk = c_attn_b[1 * C:2 * C][hs]
    bv = c_attn_b[2 * C:3 * C][hs]
    cs = slice(rk * NP, (rk + 1) * NP)

    # c_proj rows permuted to match the gathered layout: half-major,
    # then rank-major, then 2 heads x 64 dims.
    perm = []
    for half in range(HL // 2):
        for r in range(GS):
            base = (r * HL + 2 * half) * D
            perm.extend(range(base, base + 2 * D))
    wp = c_proj_w[np.array(perm)][:, cs]

    pp = np.arange(P)[:, None, None]
    jj = np.arange(4)[None, :, None]
    qq = np.arange(512)[None, None, :]
    masks = (qq >= pp + 128 * jj).astype(NP_BF16)

    bf = NP_BF16
    CC = cfg.CC

    def pack(w):
        # [C, M] -> SBUF layout [P, CC, M]: partition-contiguous packets.
        m = w.shape[1]
        return np.ascontiguousarray(
            w.reshape(CC, P, m).transpose(1, 0, 2)).astype(bf)

    xT = x_full[g, :T].astype(bf).T        # [C, T] in bf16
    xt = np.ascontiguousarray(
        xT.reshape(CC, P, cfg.TB, 512).transpose(2, 1, 0, 3))

    return {
        "xt": xt,
        "wqk": pack(np.concatenate([wq, wk], axis=1)),
        "wv": pack(wv),
        "bqk": np.ascontiguousarray(
            np.concatenate([bq, bk]).reshape(cfg.QKCH, P).T, np.float32),
        "bv": np.ascontiguousarray(bv[None, :]).astype(bf),
        "wp": pack(wp),
        "bp": np.ascontiguousarray(c_proj_b[cs][None, :]).astype(bf),
        "masks": masks,
    }


_CACHE = {}


def _build_full():
    if "nc" in _CACHE:
        return _CACHE["nc"]
    cfg = CFG_FULL
    nc = bacc.Bacc(
        "TRN2", target_bir_lowering=False, debug=False,
        num_devices=cfg.n_cores,
    )
    ins = {}
    shapes = {
        "xt": ((cfg.TB, P, cfg.CC, 512), BF16),
        "wqk": ((P, cfg.CC, cfg.QKCH * P), BF16),
        "wv": ((P, cfg.CC, cfg.HL * D), BF16),
        "bqk": ((P, cfg.QKCH), F32),
        "bv": ((1, cfg.HL * D), BF16),
        "wp": ((P, cfg.CC, cfg.NP), BF16),
        "bp": ((1, cfg.NP), BF16),
        "masks": ((P, 4, 512), BF16),
    }
    for name, (shape, dt) in shapes.items():
        ins[name] = nc.dram_tensor(
            name, list(shape), dt, kind="ExternalInput").ap()
    outs = {
        "out": nc.dram_tensor(
            "out", [cfg.T, cfg.NP], F32, kind="ExternalOutput").ap()
    }
    with tile.TileContext(nc) as tc:
        emit(tc, outs, ins, cfg)
    nc.compile()
    _CACHE["nc"] = nc
    return nc


def kernel(**inputs):
    from concourse.bass_utils import run_bass_kernel_spmd

    cfg = CFG_FULL
    x = np.asarray(inputs["x"], np.float32)
    c_attn_w = np.asarray(inputs["c_attn_w"], np.float32)
    c_attn_b = np.asarray(inputs["c_attn_b"], np.float32)
    c_proj_w = np.asarray(inputs["c_proj_w"], np.float32)
    c_proj_b = np.asarray(inputs["c_proj_b"], np.float32)

    nc = _build_full()
    in_maps = [
        make_core_inputs(x, c_attn_w, c_attn_b, c_proj_w, c_proj_b, cfg, core)
        for core in range(cfg.n_cores)
    ]
    res = run_bass_kernel_spmd(nc, in_maps, core_ids=list(range(cfg.n_cores)))
    out = np.empty((B, T_FULL, C), np.float32)
    for core in range(cfg.n_cores):
        g, rk = divmod(core, cfg.GS)
        out[g, :, rk * cfg.NP:(rk + 1) * cfg.NP] = res.results[core]["out"]
    return out
